# revision 38
# baseline (speedup 1.0000x reference)
"""Trainium2 Bass kernel for Deformable-DETR multi-scale deformable attention.

8 NeuronCores, data-parallel over batch (1 batch element per core, SPMD, no
collectives).

Per core:
  Loop1 (hoisted, runs from t=0 concurrent with phase 1): offset/attention
    projections on PE, softmax + sampling locations + masked corner weights
    + unit indices on DVE/ACT for ALL query groups.
  Phase 1: V = X @ W_v + b_v on PE (fp16 in, fp32 PSUM), stored to DRAM
    "pair tables": per (head, level), unit u = e*W + x holds rows (e-1, e)
    of column x as [2, 32ch] fp16 (128 B). A bilinear 2x2x32 patch is then
    2 consecutive units -> 2 contiguous 128B descriptors per sample.
    Table writes alternate sync/scalar HWDGE queues to halve queue latency.
  Loop2 (per 128-query group): per-slot indirect DMA gathers (128
    descriptors each, the issue-rate bottleneck), fp16 broadcast-multiply +
    add-tree combine on DVE, output projection on PE.

All potentially-junk table reads (x-wrap at row ends, out-of-range
samples) are zero-masked via the per-corner validity folded into the
bilinear weights; every reachable table byte is written or zero-filled so
junk stays finite.
"""

import sys

sys.path.insert(0, "/opt/trn_rl_repo")

import dataclasses
import math

import numpy as np

import concourse.bass as bass
import concourse.mybir as mybir
import concourse.tile as tile
from concourse import bacc
from concourse.bass_utils import run_bass_kernel_spmd
from concourse.masks import make_identity

# ---------------- problem constants (hardcoded) ----------------
SPATIAL = [(100, 150), (50, 75), (25, 38), (13, 19)]
TOTAL = sum(h * w for h, w in SPATIAL)  # 19947
BS, Q, D, NH, NL, NP = 8, 900, 256, 8, 4, 4
DH = D // NH  # 32
NQG = math.ceil(Q / 128)  # 8 query groups (7x128 + 4)
MAGIC = 12582912.0  # 1.5 * 2**23: float32 round-to-nearest-int trick
LEFTOVER = False  # slot-major tail-query fast path (crashed on HW; disabled)

FP32 = mybir.dt.float32
FP16 = mybir.dt.float16
INT32 = mybir.dt.int32
ADD = mybir.AluOpType.add
SUB = mybir.AluOpType.subtract
MUL = mybir.AluOpType.mult
MAXOP = mybir.AluOpType.max
MINOP = mybir.AluOpType.min

LVL_START = []
_s = 0
for _h, _w in SPATIAL:
    LVL_START.append(_s)
    _s += _h * _w
LVL_NT = [math.ceil(h * w / 128) for h, w in SPATIAL]
LVL_T0 = [sum(LVL_NT[:i]) for i in range(NL)]
NT_TOT = sum(LVL_NT)  # 158

# Table regions in units (1 unit = 64 fp16 = [2 rows, 32 ch] of one column)
# One DRAM table per level so gathers of a level only depend on that level's
# writes (lets small levels' gathers start while level 0 is still building).
GUARD = 64
TAILG = 384  # absorbs the last head's masked x/y-slop reads
# +1 virtual tile per head: units [nt*128, (nt+1)*128) hold [v[u-W] | 0] so
# valid bottom-row (y0 = H-1) A-corners past nt*128 read real data.
REG = [(LVL_NT[_li] + 1) * 128 for _li in range(NL)]
# masked slack reads past a head's region land in the next head's written
# units (finite, zero-weighted) -- only the last head needs the tail guard.
BASE = [[GUARD + h * REG[li] for h in range(NH)] for li in range(NL)]  # [l][h]
U_L = [GUARD + NH * REG[li] + TAILG for li in range(NL)]
# xf16 staging layout: per-level, padded to 1024-row bands
LVL_NB = [math.ceil(nt * 128 / 1024) for nt in LVL_NT]
XB = [sum(LVL_NB[:i]) * 1024 for i in range(NL)]
XROWS = sum(LVL_NB) * 1024


def _np_base_const() -> np.ndarray:
    """[128, 128] f32; free position (h,l,p) -> BASE[l][h] + W_l (level-local)."""
    c = np.zeros((128,), np.float32)
    for h in range(NH):
        for li in range(NL):
            for p in range(NP):
                c[(h * NL + li) * NP + p] = BASE[li][h] + SPATIAL[li][1]
    return np.broadcast_to(c, (128, 128)).copy()


def _bc(ap, dim, count):
    """Insert a broadcast (step-0) dim at position `dim` of an AP."""
    new = list(ap.ap)
    new.insert(dim, [0, count])
    return dataclasses.replace(ap, ap=new)


def _indirect_gather_q(gp, out, in_, offset_ap, queue_num: int):
    """indirect_dma_start (gather direction only) with SWDGE queue select."""
    out_ap = gp.lower_ap_dma(out, for_indirect_dma=True)
    in_ap = gp.lower_ap_dma(in_, for_indirect_dma=True)
    assert len(in_ap) == 1 and len(out_ap) == 1
    off_ap = gp.lower_ap_dma(offset_ap)
    assert len(off_ap) == 1
    in_ap.append(off_ap[0])
    ap_shape = in_.shape
    coef = 1
    for i in range(1, len(ap_shape)):
        coef *= ap_shape[i]
    in_ap[0].dynamic_ap_info = mybir.DynamicAccessPatternInfo(
        c=0,
        actual_ap=out.ap,
        indirect_dim_max_index=ap_shape[0],
        offset_expr=[
            mybir.DynamicAccessPatternOffsetExpr(
                coef=coef,
                aff_expr=mybir.DynamicAccessPatternOffsetExprAffExpr(
                    kind="IndirectArgId", arg_id=1
                ),
            )
        ],
    )
    return gp.add_instruction(
        mybir.InstDMACopy(
            name=gp.bass.get_next_instruction_name(),
            queue=f"qPoolDynamic{queue_num or ''}",
            mode="Copy",
            ins=in_ap,
            outs=out_ap,
            oob_is_err=True,
            cce_op=mybir.AluOpType.bypass,
        )
    )


def build(reps: int = 1, ablate: set | None = None):
    ablate = ablate or set()
    nc = bacc.Bacc(
        "TRN2", target_bir_lowering=False, debug=False, num_devices=8,
    )

    t_q = nc.dram_tensor("query", [Q, D], FP32, kind="ExternalInput")
    t_rp = nc.dram_tensor("reference_points", [Q, NL * 2], FP32, kind="ExternalInput")
    t_x = nc.dram_tensor("input_flatten", [TOTAL, D], FP32, kind="ExternalInput")
    t_woff = nc.dram_tensor("W_off", [D, D], FP32, kind="ExternalInput")
    t_boff = nc.dram_tensor("b_off", [D], FP32, kind="ExternalInput")
    t_watt = nc.dram_tensor("W_attn", [D, 128], FP32, kind="ExternalInput")
    t_batt = nc.dram_tensor("b_attn", [128], FP32, kind="ExternalInput")
    t_wv = nc.dram_tensor("W_v", [D, D], FP32, kind="ExternalInput")
    t_bv = nc.dram_tensor("b_v", [D], FP32, kind="ExternalInput")
    t_wo = nc.dram_tensor("W_o", [D, D], FP32, kind="ExternalInput")
    t_bo = nc.dram_tensor("b_o", [D], FP32, kind="ExternalInput")
    t_bconst = nc.dram_tensor("base_const", [128, 128], FP32, kind="ExternalInput")
    t_out = nc.dram_tensor("out", [Q, D], FP32, kind="ExternalOutput")

    with tile.TileContext(nc) as tc:
        with (
            tc.tile_pool(name="dram", bufs=1, space="DRAM") as dpool,
            tc.tile_pool(name="const", bufs=1) as cpool,
            tc.tile_pool(name="psum", bufs=2, space="PSUM") as ppool,
            tc.tile_pool(name="psum2", bufs=1, space="PSUM") as ppool2,
        ):
            tables = []
            for _tli in range(NL):
                _tab = dpool.tile([U_L[_tli], 64], FP16, tag=f"table{_tli}")
                tables.append(_tab)

            # ---- constants / weights ----
            ident = cpool.tile([128, 128], FP32)
            make_identity(nc, ident[:])
            ident16 = cpool.tile([128, 128], FP16)
            nc.vector.tensor_copy(ident16[:], ident[:])
            # Per-level shift matrices for the table A-half (v shifted by W):
            # Sh[k, m] = [m == k + k1], Sl[k, m] = [m == k - (128 - k1)].
            shmat, slmat = {}, {}
            for _li in range(NL):
                _k1 = SPATIAL[_li][1] % 128
                _sh = cpool.tile([128, 128], FP16, tag=f"sh{_li}")
                _sl = cpool.tile([128, 128], FP16, tag=f"sl{_li}")
                nc.vector.memset(_sh[:], 0.0)
                nc.vector.memset(_sl[:], 0.0)
                nc.vector.tensor_copy(_sh[:, _k1:128], ident16[:, 0 : 128 - _k1])
                nc.vector.tensor_copy(_sl[:, 0:_k1], ident16[:, 128 - _k1 : 128])
                shmat[_li], slmat[_li] = _sh, _sl
            wv16 = cpool.tile([128, 2, D], FP16)
            wo16 = cpool.tile([128, 2, D], FP16)
            woff = cpool.tile([128, 2, D], FP32)
            watt = cpool.tile([128, 2, 128], FP32)
            for j in range(2):
                nc.gpsimd.dma_start(out=wv16[:, j, :], in_=t_wv[j * 128 : (j + 1) * 128, :])
                nc.gpsimd.dma_start(out=wo16[:, j, :], in_=t_wo[j * 128 : (j + 1) * 128, :])
                nc.sync.dma_start(out=woff[:, j, :], in_=t_woff[j * 128 : (j + 1) * 128, :])
                nc.sync.dma_start(out=watt[:, j, :], in_=t_watt[j * 128 : (j + 1) * 128, :])
            bv16 = cpool.tile([1, D], FP16)
            bo16 = cpool.tile([1, D], FP16)
            boff = cpool.tile([1, D], FP32)
            batt = cpool.tile([1, 128], FP32)
            nc.gpsimd.dma_start(out=bv16[:], in_=t_bv[None, :])
            nc.gpsimd.dma_start(out=bo16[:], in_=t_bo[None, :])
            nc.sync.dma_start(out=boff[:], in_=t_boff[None, :])
            nc.sync.dma_start(out=batt[:], in_=t_batt[None, :])
            ones32 = cpool.tile([1, 128], FP32)
            ones16 = cpool.tile([1, 128], FP16)
            nc.vector.memset(ones32[:], 1.0)
            nc.vector.memset(ones16[:], 1.0)
            bconst = cpool.tile([128, 128], FP32)
            nc.sync.dma_start(out=bconst[:], in_=t_bconst[:, :])

            # zero tile for table guard/slack fills
            zt = cpool.tile([128, 64], FP16)
            nc.vector.memset(zt[:], 0.0)

            # one-hot head map for the leftover-query combine matmul:
            # slot s (level-major: s = l*32 + h*4 + p) -> head h.
            # oneh[s, h] = sum_{l,p} ident[s, l*32 + h*4 + p], via two reduces.
            oneh = cpool.tile([128, NH], FP16) if LEFTOVER else None
            onehr = cpool.tile([128, NL * NH], FP16) if LEFTOVER else None
            if LEFTOVER:
             with nc.allow_low_precision(reason="0/1 one-hot sums are exact in fp16"):
                nc.vector.tensor_reduce(
                    onehr[:, :].rearrange("s (l h) -> s l h", l=NL),
                    ident16[:, :].rearrange("s (l h p) -> s l h p", l=NL, h=NH),
                    mybir.AxisListType.X, ADD,
                )
                nc.vector.tensor_reduce(
                    oneh[:, :],
                    onehr[:, :].rearrange("s (l h) -> s h l", l=NL),
                    mybir.AxisListType.X, ADD,
                )

            for rep in range(reps):
              if rep:
                  tc.no_sync_barrier()
              with (
                  tc.tile_pool(name=f"p2w{rep}", bufs=2) as wp2,
                  tc.tile_pool(name=f"p2s{rep}", bufs=1) as sp2,
                  tc.tile_pool(name=f"p2b{rep}", bufs=1) as bigp,
                  tc.tile_pool(name=f"p2p{rep}", bufs=3) as patpool,
                  tc.tile_pool(name=f"p2i{rep}", bufs=8) as ipool,
              ):
                # ======== Loop1: sampling indices + weights for ALL qgs =====
                # Runs concurrently with phase 1 (only needs query + weights).
                idx_t, cw_t = {}, {}
                idxT7_t, cwT7_t = {}, {}
                for qg in range(NQG):
                    q0 = qg * 128
                    nq = min(128, Q - q0)

                    qt32 = wp2.tile([128, D], FP32, tag="qt32")
                    nc.sync.dma_start(out=qt32[:nq, :], in_=t_q[q0 : q0 + nq, :])
                    qT = wp2.tile([128, 2, 128], FP32, tag="qT")
                    for j in range(2):
                        tp = ppool2.tile([128, 128], FP32, tag="tpsum")
                        nc.tensor.transpose(
                            tp[:, :nq], qt32[:nq, j * 128 : (j + 1) * 128], ident[:nq, :nq]
                        )
                        nc.scalar.copy(out=qT[:, j, :nq], in_=tp[:, :nq])

                    offp = ppool2.tile([128, D], FP32, tag="mmout2")
                    nc.tensor.matmul(offp[:nq, :], qT[:, 0, :nq], woff[:, 0, :], start=True, stop=False)
                    nc.tensor.matmul(offp[:nq, :], qT[:, 1, :nq], woff[:, 1, :], start=False, stop=False)
                    nc.tensor.matmul(offp[:nq, :], ones32[:, :nq], boff[:], start=False, stop=True)
                    off = sp2.tile([128, D], FP32, tag="off")
                    nc.scalar.copy(out=off[:nq, :], in_=offp[:nq, :])

                    attp = ppool2.tile([128, 128], FP32, tag="attp")
                    nc.tensor.matmul(attp[:nq, :], qT[:, 0, :nq], watt[:, 0, :], start=True, stop=False)
                    nc.tensor.matmul(attp[:nq, :], qT[:, 1, :nq], watt[:, 1, :], start=False, stop=False)
                    nc.tensor.matmul(attp[:nq, :], ones32[:, :nq], batt[:], start=False, stop=True)
                    att = sp2.tile([128, 128], FP32, tag="att")
                    nc.scalar.copy(out=att[:nq, :], in_=attp[:nq, :])

                    # softmax over (l,p)=16 per head
                    mx = sp2.tile([128, 1], FP32, tag="mx")
                    nc.vector.tensor_reduce(mx[:nq, :], att[:nq, :], mybir.AxisListType.X, MAXOP)
                    nmx = sp2.tile([128, 1], FP32, tag="nmx")
                    nc.vector.tensor_scalar_mul(nmx[:nq, :], mx[:nq, :], -1.0)
                    ex = sp2.tile([128, 128], FP32, tag="ex")
                    nc.scalar.activation(
                        ex[:nq, :], att[:nq, :], mybir.ActivationFunctionType.Exp,
                        bias=nmx[:nq, :], scale=1.0,
                    )
                    s16 = sp2.tile([128, 8], FP32, tag="s16")
                    nc.vector.tensor_reduce(
                        s16[:nq, :], ex[:nq, :].rearrange("q (h k) -> q h k", k=16),
                        mybir.AxisListType.X, ADD,
                    )
                    r16 = sp2.tile([128, 8], FP32, tag="r16")
                    nc.vector.reciprocal(r16[:nq, :], s16[:nq, :])
                    attn = sp2.tile([128, 128], FP32, tag="attn")
                    nc.vector.tensor_tensor(
                        attn[:nq, :].rearrange("q (h k) -> q h k", k=16),
                        ex[:nq, :].rearrange("q (h k) -> q h k", k=16),
                        _bc(r16[:nq, :], 2, 16),
                        MUL,
                    )

                    # reference points -> pixel bases per (l, p)
                    rxy = sp2.tile([128, 8], FP32, tag="rxy")
                    nc.sync.dma_start(out=rxy[:nq, :], in_=t_rp[q0 : q0 + nq, :])
                    refx = sp2.tile([128, 16], FP32, tag="refx")
                    refy = sp2.tile([128, 16], FP32, tag="refy")
                    for li, (H, W) in enumerate(SPATIAL):
                        nc.vector.tensor_scalar(
                            refx[:nq, li * 4 : li * 4 + 4],
                            rxy[:nq, 2 * li : 2 * li + 1].to_broadcast([nq, 4]),
                            float(W), -0.5, MUL, ADD,
                        )
                        nc.vector.tensor_scalar(
                            refy[:nq, li * 4 : li * 4 + 4],
                            rxy[:nq, 2 * li + 1 : 2 * li + 2].to_broadcast([nq, 4]),
                            float(H), -0.5, MUL, ADD,
                        )

                    xc = sp2.tile([128, 128], FP32, tag="xc")
                    yc = sp2.tile([128, 128], FP32, tag="yc")
                    off_v = off[:nq, :].rearrange(
                        "q (h l p two) -> q h l p two", h=NH, l=NL, p=NP
                    )
                    nc.vector.tensor_tensor(
                        xc[:nq, :].rearrange("q (h lp) -> q h lp", h=NH),
                        off_v[:, :, :, :, 0].rearrange("q h l p -> q h (l p)"),
                        _bc(refx[:nq, :], 1, NH),
                        ADD,
                    )
                    nc.vector.tensor_tensor(
                        yc[:nq, :].rearrange("q (h lp) -> q h lp", h=NH),
                        off_v[:, :, :, :, 1].rearrange("q h l p -> q h (l p)"),
                        _bc(refy[:nq, :], 1, NH),
                        ADD,
                    )

                    # floor via magic round + correction, then per-level clamp
                    def floor_clamp(src, tagp, hi_by_l):
                        f = sp2.tile([128, 128], FP32, tag="f" + tagp)
                        nc.vector.tensor_scalar_add(f[:nq, :], src[:nq, :], MAGIC)
                        nc.vector.tensor_scalar_sub(f[:nq, :], f[:nq, :], MAGIC)
                        g = sp2.tile([128, 128], FP32, tag="g" + tagp)
                        nc.vector.tensor_tensor(g[:nq, :], f[:nq, :], src[:nq, :], mybir.AluOpType.is_gt)
                        nc.vector.tensor_tensor(f[:nq, :], f[:nq, :], g[:nq, :], SUB)
                        fv = f[:nq, :].rearrange("q (h l p) -> q h l p", h=NH, l=NL)
                        for li in range(NL):
                            nc.vector.tensor_scalar(
                                fv[:, :, li, :], fv[:, :, li, :],
                                -2.0, float(hi_by_l[li]), MAXOP, MINOP,
                            )
                        return f

                    x0f = floor_clamp(xc, "x", [w for (h, w) in SPATIAL])
                    y0f = floor_clamp(yc, "y", [h for (h, w) in SPATIAL])

                    wx1 = sp2.tile([128, 128], FP32, tag="wx1")
                    wy1 = sp2.tile([128, 128], FP32, tag="wy1")
                    nc.vector.tensor_tensor(wx1[:nq, :], xc[:nq, :], x0f[:nq, :], SUB)
                    nc.vector.tensor_tensor(wy1[:nq, :], yc[:nq, :], y0f[:nq, :], SUB)

                    def corner_w(wf1, f, axis, n_by_l):
                        a0 = sp2.tile([128, 128], FP32, tag="a0" + axis)
                        a1 = sp2.tile([128, 128], FP32, tag="a1" + axis)
                        m = sp2.tile([128, 128], FP32, tag="m" + axis)
                        nc.vector.tensor_scalar(a0[:nq, :], wf1[:nq, :], -1.0, 1.0, MUL, ADD)
                        nc.vector.tensor_scalar(m[:nq, :], f[:nq, :], 0.0, None, mybir.AluOpType.is_ge)
                        nc.vector.tensor_tensor(a0[:nq, :], a0[:nq, :], m[:nq, :], MUL)
                        nc.vector.tensor_scalar(m[:nq, :], f[:nq, :], -1.0, None, mybir.AluOpType.is_ge)
                        nc.vector.tensor_tensor(a1[:nq, :], wf1[:nq, :], m[:nq, :], MUL)
                        fv = f[:nq, :].rearrange("q (h l p) -> q h l p", h=NH, l=NL)
                        mv = m[:nq, : NH * NP].rearrange("q (h p) -> q h p", h=NH)
                        for li in range(NL):
                            n = n_by_l[li]
                            for a, bound in ((a0, n - 1.0), (a1, n - 2.0)):
                                nc.vector.tensor_scalar(mv, fv[:, :, li, :], bound, None, mybir.AluOpType.is_le)
                                av = a[:nq, :].rearrange("q (h l p) -> q h l p", h=NH, l=NL)
                                nc.vector.tensor_tensor(av[:, :, li, :], av[:, :, li, :], mv, MUL)
                        return a0, a1

                    ax0, ax1 = corner_w(wx1, x0f, "x", [w for (h, w) in SPATIAL])
                    ay0, ay1 = corner_w(wy1, y0f, "y", [h for (h, w) in SPATIAL])
                    nc.vector.tensor_tensor(ay0[:nq, :], ay0[:nq, :], attn[:nq, :], MUL)
                    nc.vector.tensor_tensor(ay1[:nq, :], ay1[:nq, :], attn[:nq, :], MUL)

                    cw = ipool.tile([128, 128, 4], FP16, tag="cw")
                    for dx, ax in ((0, ax0), (1, ax1)):
                        for dy, ay in ((0, ay0), (1, ay1)):
                            nc.vector.tensor_tensor(
                                cw[:nq, :, 2 * dx + dy], ax[:nq, :], ay[:nq, :], MUL
                            )

                    # u = (y0+1)*W + x0 + BASE  (bconst = BASE + W)
                    uf = sp2.tile([128, 128], FP32, tag="uf")
                    ufv = uf[:nq, :].rearrange("q (h l p) -> q h l p", h=NH, l=NL)
                    yv = y0f[:nq, :].rearrange("q (h l p) -> q h l p", h=NH, l=NL)
                    for li in range(NL):
                        nc.vector.tensor_scalar(
                            ufv[:, :, li, :], yv[:, :, li, :],
                            float(SPATIAL[li][1]), None, MUL,
                        )
                    nc.vector.tensor_tensor(uf[:nq, :], uf[:nq, :], x0f[:nq, :], ADD)
                    nc.vector.tensor_tensor(uf[:nq, :], uf[:nq, :], bconst[:nq, :], ADD)
                    for li in range(NL):
                        nc.vector.tensor_scalar(
                            ufv[:, :, li, :], ufv[:, :, li, :],
                            0.0, float(U_L[li] - 2), MAXOP, MINOP,
                        )

                    idx = ipool.tile([128, 128], INT32, tag="idx")
                    nc.vector.tensor_copy(idx[:nq, :], uf[:nq, :])

                    idx_t[qg] = idx
                    cw_t[qg] = cw

                    if LEFTOVER and qg == NQG - 1 and nq <= 8:
                        # Leftover-query path: transpose indices and corner
                        # weights to slot-major (level-major slot order) so the
                        # tail queries gather 32 slots per instruction.
                        def tposeL(src, tag):
                            perm = sp2.tile([128, 128], FP32, tag="permT")
                            nc.vector.tensor_copy(
                                perm[:nq, :].rearrange("q (l h p) -> q l h p", l=NL, h=NH),
                                src[:nq, :].rearrange("q (h l p) -> q l h p", h=NH, l=NL),
                            )
                            tp7 = ppool2.tile([128, 128], FP32, tag="tpsum")
                            nc.tensor.transpose(tp7[:, :nq], perm[:nq, :], ident[:nq, :nq])
                            dst = sp2.tile([128, 8], FP32, tag="T" + tag)
                            nc.scalar.copy(out=dst[:, :nq], in_=tp7[:, :nq])
                            return dst

                        ufT = tposeL(uf, "uf")
                        idxT7 = ipool.tile([128, 8], INT32, tag="idxT7")
                        nc.vector.tensor_copy(idxT7[:, :nq], ufT[:, :nq])
                        a0xT = tposeL(ax0, "a0x")
                        a1xT = tposeL(ax1, "a1x")
                        a0yT = tposeL(ay0, "a0y")
                        a1yT = tposeL(ay1, "a1y")
                        cwT7 = ipool.tile([128, 8, 4], FP16, tag="cwT7")
                        for dx, axT in ((0, a0xT), (1, a1xT)):
                            for dy, ayT in ((0, a0yT), (1, a1yT)):
                                nc.vector.tensor_tensor(
                                    cwT7[:, :nq, 2 * dx + dy], axT[:, :nq], ayT[:, :nq], MUL
                                )
                        idxT7_t[0] = idxT7
                        cwT7_t[0] = cwT7

                # ---- zero-fill guards + unwritten slack (sync/scalar split) --
                _zq = [0]

                def zfill(tab, u0, n):
                    while n > 0:
                        k = min(n, 128)
                        eng = nc.sync if (_zq[0] & 1) == 0 else nc.scalar
                        _zq[0] += 1
                        eng.dma_start(out=tab[u0 : u0 + k, :], in_=zt[:k, :])
                        u0 += k
                        n -= k

                if rep == 0:
                    for li in (3, 2, 1, 0):
                        # head guard + tail guard; every unit in [0, nt*128)
                        # of every head is fully written by the merged A|B
                        # table writes (A-half zeros where pos-W < 0).
                        zfill(tables[li], 0, GUARD)
                        zfill(tables[li], U_L[li] - TAILG, TAILG)

                # ============ Phase 1: value projection -> pair tables ======
                CHUNK_BANDS = 3  # 24(+1) tiles per vcat chunk
                with (
                    tc.tile_pool(name=f"vsb{rep}", bufs=2) as vpool,
                    tc.tile_pool(name=f"p1w{rep}", bufs=3) as wp1,
                    tc.tile_pool(name=f"p1d{rep}", bufs=1, space="DRAM") as dp1,
                ):
                    if "phase1" not in ablate:
                        # per-level fp16 copy of X in DRAM (cast during DMA),
                        # processed smallest level first so its tables finish
                        # early and loop2 gathers can start.
                        xf16 = dp1.tile([XROWS, D], FP16)
                        zrow = wp1.tile([128, D], FP16, tag="zrow")
                        nc.vector.memset(zrow[:, :], 0.0)
                        for li in (3, 2, 1, 0):
                            H, W = SPATIAL[li]
                            npos = H * W
                            p0 = XB[li]
                            k1 = W % 128  # partition shift of the A-half
                            tb = W // 128  # whole-tile shift of the A-half
                            nc.gpsimd.dma_start(
                                out=xf16[p0 : p0 + npos, :],
                                in_=t_x[LVL_START[li] : LVL_START[li] + npos, :],
                            )
                            r = p0 + npos
                            while r < p0 + LVL_NB[li] * 1024:
                                k = min(128, p0 + LVL_NB[li] * 1024 - r)
                                nc.scalar.dma_start(out=xf16[r : r + k, :], in_=zrow[:k, :])
                                r += k
                            prev_vcat, prev_tc0 = None, -1
                            for c0 in range(0, LVL_NB[li], CHUNK_BANDS):
                                cbands = min(CHUNK_BANDS, LVL_NB[li] - c0)
                                tc0 = c0 * 8  # first tile (level-local)
                                # +1: the virtual bottom tile (A-only) rides in
                                # the last chunk of the level.
                                is_last = c0 + cbands >= LVL_NB[li]
                                cap = cbands * 8 + (1 if is_last else 0)
                                ntc = min(cap, LVL_NT[li] + 1 - tc0)
                                assert ntc <= CHUNK_BANDS * 8 + 1
                                vcat = vpool.tile(
                                    [128, CHUNK_BANDS * 8 + 1, NH, 2 * DH], FP16, tag="vcat"
                                )

                                def bsrc(tt, lo, hi):
                                    # B-half (v) of level-local tile tt, partitions lo:hi
                                    if tt >= tc0:
                                        return vcat[lo:hi, tt - tc0, :, DH : 2 * DH]
                                    return prev_vcat[lo:hi, tt - prev_tc0, :, DH : 2 * DH]

                                for bloc in range(cbands):
                                    band = c0 + bloc
                                    xTb = wp1.tile([128, 2, 1024], FP16, tag="xTb")
                                    for j in range(2):
                                        nc.sync.dma_start_transpose(
                                            out=xTb[:, j, :],
                                            in_=xf16[p0 + band * 1024 : p0 + (band + 1) * 1024, j * 128 : (j + 1) * 128],
                                        )
                                    for tloc in range(8):
                                        t = band * 8 + tloc
                                        if t >= LVL_NT[li]:
                                            break
                                        vp = ppool.tile([128, D], FP32, tag="mmout")
                                        nc.tensor.matmul(vp[:], xTb[:, 0, tloc * 128 : (tloc + 1) * 128], wv16[:, 0, :], start=True, stop=False)
                                        nc.tensor.matmul(vp[:], xTb[:, 1, tloc * 128 : (tloc + 1) * 128], wv16[:, 1, :], start=False, stop=False)
                                        nc.tensor.matmul(vp[:], ones16[:, :], bv16[:], start=False, stop=True)
                                        nc.scalar.copy(
                                            out=vcat[:, t - tc0, :, DH : 2 * DH],
                                            in_=vp[:].rearrange("p (h c) -> p h c", h=NH),
                                        )
                                        # A-half: v shifted back W positions
                                        # (unit u holds [v[u-W] | v[u]]).
                                        pa = ppool.tile([128, D], FP32, tag="mmoutA")
                                        if t - tb >= 0:
                                            more = t - tb - 1 >= 0
                                            nc.tensor.matmul(
                                                pa[:, :], shmat[li][:, :],
                                                bsrc(t - tb, 0, 128),
                                                start=True, stop=not more,
                                            )
                                            if more:
                                                nc.tensor.matmul(
                                                    pa[:, :], slmat[li][:, :],
                                                    bsrc(t - tb - 1, 0, 128),
                                                    start=False, stop=True,
                                                )
                                        else:
                                            nc.vector.memset(pa[:, :], 0.0)
                                        nc.scalar.copy(
                                            out=vcat[:, t - tc0, :, 0:DH],
                                            in_=pa[:].rearrange("p (h c) -> p h c", h=NH),
                                        )
                                t_v = LVL_NT[li]  # virtual bottom tile
                                if tc0 <= t_v < tc0 + cap:
                                    nc.vector.memset(
                                        vcat[:, t_v - tc0, :, DH : 2 * DH], 0.0
                                    )
                                    pa = ppool.tile([128, D], FP32, tag="mmoutA")
                                    nc.tensor.matmul(
                                        pa[:, :], shmat[li][:, :],
                                        bsrc(t_v - tb, 0, 128),
                                        start=True, stop=False,
                                    )
                                    nc.tensor.matmul(
                                        pa[:, :], slmat[li][:, :],
                                        bsrc(t_v - tb - 1, 0, 128),
                                        start=False, stop=True,
                                    )
                                    nc.scalar.copy(
                                        out=vcat[:, t_v - tc0, :, 0:DH],
                                        in_=pa[:].rearrange("p (h c) -> p h c", h=NH),
                                    )
                                u0 = tc0 * 128
                                for h in range(NH):
                                    b = BASE[li][h]
                                    src = vcat[:, 0:ntc, h, :]
                                    dst = tables[li][b + u0 : b + u0 + ntc * 128, :].rearrange(
                                        "(t p) c -> p t c", p=128
                                    )
                                    eng = nc.sync if (h & 1) == 0 else nc.scalar
                                    eng.dma_start(out=dst, in_=src)
                                prev_vcat, prev_tc0 = vcat, tc0

                # ============ Loop2: gather + combine + output proj =========
                # Level-0 gathers + combine for qg k are emitted after qg
                # k+1's small-level gathers, so the (late-finishing) level-0
                # table writes get an extra ~130us of gather work as cover.
                patches_t = {}

                def emit_smalls(qg):
                    q0 = qg * 128
                    nq = min(128, Q - q0)
                    idx = idx_t[qg]
                    # ---- gather: one 256B patch per partition per instr ----
                    # (HW indirect DMA semantics: one index per partition,
                    #  contiguous out-row-sized read.)
                    patches = patpool.tile([128, 16384], FP16, tag="patches")
                    patches_t[qg] = patches
                    pview = patches[:nq, :].rearrange("q (s e) -> q s e", e=128)
                    if "gather" in ablate:
                        nc.vector.memset(patches[:, :], 0.0)
                        return
                    for li in (3, 2, 1):
                        for h in range(NH):
                            for p in range(NP):
                                s = (h * NL + li) * NP + p
                                _indirect_gather_q(
                                    nc.gpsimd,
                                    pview[:, s, :],
                                    tables[li][:, :],
                                    idx[:nq, s : s + 1],
                                    queue_num=0,
                                )

                def emit_l0_combine(qg):
                    q0 = qg * 128
                    nq = min(128, Q - q0)
                    idx = idx_t[qg]
                    cw = cw_t[qg]
                    patches = patches_t.pop(qg)
                    pview = patches[:nq, :].rearrange("q (s e) -> q s e", e=128)
                    if "gather" not in ablate:
                        li = 0
                        for h in range(NH):
                            for p in range(NP):
                                s = (h * NL + li) * NP + p
                                _indirect_gather_q(
                                    nc.gpsimd,
                                    pview[:, s, :],
                                    tables[li][:, :],
                                    idx[:nq, s : s + 1],
                                    queue_num=0,
                                )

                    # ---- combine ----
                    if "combine" in ablate:
                        og = wp2.tile([128, D], FP16, tag="og")
                        nc.vector.memset(og[:, :], 0.0)
                    if "combine" not in ablate:
                        # weight multiply in place (out == in1, elementwise)
                        nc.vector.tensor_tensor(
                            patches[:nq, :].rearrange("q (s c ch) -> q s c ch", s=128, c=4),
                            patches[:nq, :].rearrange("q (s c ch) -> q s c ch", s=128, c=4),
                            _bc(cw[:nq, :, :], 3, DH),
                            MUL,
                        )
                        # add-tree fully in place inside patches (out == in1
                        # elementwise), so the tile frees 31KB of tree scratch
                        # and patches gets a 3-deep ring.
                        pmv = patches[:nq, :].rearrange("q (s d) -> q s d", d=128)
                        nc.vector.tensor_tensor(pmv[:, :, 0:64], pmv[:, :, 0:64], pmv[:, :, 64:128], ADD)
                        nc.vector.tensor_tensor(pmv[:, :, 0:32], pmv[:, :, 0:32], pmv[:, :, 32:64], ADD)
                        sv = patches[:nq, :].rearrange("q (h k d) -> q h k d", h=NH, k=16)
                        nc.vector.tensor_tensor(sv[:, :, 0:8, 0:32], sv[:, :, 0:8, 0:32], sv[:, :, 8:16, 0:32], ADD)
                        nc.vector.tensor_tensor(sv[:, :, 0:4, 0:32], sv[:, :, 0:4, 0:32], sv[:, :, 4:8, 0:32], ADD)
                        nc.vector.tensor_tensor(sv[:, :, 0:2, 0:32], sv[:, :, 0:2, 0:32], sv[:, :, 2:4, 0:32], ADD)
                        og = wp2.tile([128, D], FP16, tag="og")
                        if nq < 128:
                            nc.vector.memset(og[:, :], 0.0)
                        nc.vector.tensor_tensor(
                            og[:nq, :].rearrange("q (h ch) -> q h ch", h=NH),
                            sv[:, :, 0, 0:32], sv[:, :, 1, 0:32], ADD,
                        )

                    # ---- output projection ----
                    ogT = wp2.tile([128, 2, 128], FP16, tag="ogT")
                    for j in range(2):
                        nc.scalar.dma_start_transpose(
                            out=ogT[:, j, :], in_=og[:, j * 128 : (j + 1) * 128]
                        )
                    outp = ppool2.tile([128, D], FP32, tag="mmout2")
                    nc.tensor.matmul(outp[:nq, :], ogT[:, 0, :nq], wo16[:, 0, :], start=True, stop=False)
                    nc.tensor.matmul(outp[:nq, :], ogT[:, 1, :nq], wo16[:, 1, :], start=False, stop=False)
                    nc.tensor.matmul(outp[:nq, :], ones16[:, :nq], bo16[:], start=False, stop=True)
                    ofin = wp2.tile([128, D], FP32, tag="ofin")
                    nc.scalar.copy(out=ofin[:nq, :], in_=outp[:nq, :])
                    nc.scalar.dma_start(out=t_out[q0 : q0 + nq, :], in_=ofin[:nq, :])

                def emit_leftover(qg):
                    # Tail queries (nq <= 8): slot-major gathers -- one
                    # instruction per (query, level) with slots on partitions,
                    # combine via one-hot head matmul + c-reduce.
                    q0 = qg * 128
                    nq = min(128, Q - q0)
                    idxT7 = idxT7_t[0]
                    cwT7 = cwT7_t[0]
                    p7 = bigp.tile([128, 8, 128], FP16, tag="p7")
                    if "gather" not in ablate:
                        for q in range(nq):
                            for li in (3, 2, 1, 0):
                                _indirect_gather_q(
                                    nc.gpsimd,
                                    p7[li * 32 : (li + 1) * 32, q, :],
                                    tables[li][:, :],
                                    idxT7[li * 32 : (li + 1) * 32, q : q + 1],
                                    queue_num=0,
                                )
                    else:
                        nc.vector.memset(p7[:, :, :], 0.0)
                    # weight multiply in place: [s, q, c, ch] x cwT[s, q, c]
                    nc.vector.tensor_tensor(
                        p7[:, 0:nq, :].rearrange("s q (c ch) -> s q c ch", c=4),
                        p7[:, 0:nq, :].rearrange("s q (c ch) -> s q c ch", c=4),
                        _bc(cwT7[:, 0:nq, :], 3, DH),
                        MUL,
                    )
                    # sum over slots per head: psum[h, (q c ch)]
                    ps7 = ppool2.tile([128, 512], FP32, tag="ps7")
                    nc.tensor.matmul(
                        ps7[:NH, : nq * 128],
                        oneh[:, :],
                        p7[:, 0:nq, :].rearrange("s q e -> s (q e)"),
                        start=True, stop=True,
                    )
                    # reduce the 4 corners: stage[h, q, ch]
                    st7 = sp2.tile([128, 8 * DH], FP16, tag="st7")
                    with nc.allow_low_precision(
                        reason="4-corner fp16 sum matches the main path's fp16 add-tree"
                    ):
                        nc.vector.tensor_reduce(
                            st7[:NH, : nq * DH].rearrange("h (q ch) -> h q ch", ch=DH),
                            ps7[:NH, : nq * 128].rearrange(
                                "h (q c ch) -> h q ch c", c=4, ch=DH
                            ),
                            mybir.AxisListType.X, ADD,
                        )
                    og = wp2.tile([128, D], FP16, tag="og")
                    nc.vector.memset(og[:, :], 0.0)
                    for q in range(nq):
                        # partition-fold: [8 heads, 32ch] -> one 256-wide row
                        nc.sync.dma_start(
                            out=og[q : q + 1, :].rearrange("o (h ch) -> o h ch", h=NH),
                            in_=st7[:NH, q * DH : (q + 1) * DH],
                        )
                    # ---- output projection (same as main path) ----
                    ogT = wp2.tile([128, 2, 128], FP16, tag="ogT")
                    for j in range(2):
                        nc.scalar.dma_start_transpose(
                            out=ogT[:, j, :], in_=og[:, j * 128 : (j + 1) * 128]
                        )
                    outp = ppool2.tile([128, D], FP32, tag="mmout2")
                    nc.tensor.matmul(outp[:nq, :], ogT[:, 0, :nq], wo16[:, 0, :], start=True, stop=False)
                    nc.tensor.matmul(outp[:nq, :], ogT[:, 1, :nq], wo16[:, 1, :], start=False, stop=False)
                    nc.tensor.matmul(outp[:nq, :], ones16[:, :nq], bo16[:], start=False, stop=True)
                    ofin = wp2.tile([128, D], FP32, tag="ofin")
                    nc.scalar.copy(out=ofin[:nq, :], in_=outp[:nq, :])
                    nc.scalar.dma_start(out=t_out[q0 : q0 + nq, :], in_=ofin[:nq, :])

                pend = []
                nmain = NQG - 1 if LEFTOVER else NQG
                for qg in range(nmain):
                    emit_smalls(qg)
                    pend.append(qg)
                    if len(pend) > 2:
                        emit_l0_combine(pend.pop(0))
                while pend:
                    emit_l0_combine(pend.pop(0))
                if LEFTOVER:
                    emit_leftover(NQG - 1)

    nc.compile()
    return nc


_NC_CACHE = None


def kernel(**inputs) -> np.ndarray:
    global _NC_CACHE
    if _NC_CACHE is None:
        _NC_CACHE = build()
    nc = _NC_CACHE
    bconst = _np_base_const()
    in_maps = []
    for b in range(BS):
        in_maps.append(
            {
                "query": np.ascontiguousarray(inputs["query"][b], np.float32),
                "reference_points": np.ascontiguousarray(
                    inputs["reference_points"][b], np.float32
                ).reshape(Q, NL * 2),
                "input_flatten": np.ascontiguousarray(inputs["input_flatten"][b], np.float32),
                "W_off": np.ascontiguousarray(inputs["W_off"], np.float32),
                "b_off": np.ascontiguousarray(inputs["b_off"], np.float32),
                "W_attn": np.ascontiguousarray(inputs["W_attn"], np.float32),
                "b_attn": np.ascontiguousarray(inputs["b_attn"], np.float32),
                "W_v": np.ascontiguousarray(inputs["W_v"], np.float32),
                "b_v": np.ascontiguousarray(inputs["b_v"], np.float32),
                "W_o": np.ascontiguousarray(inputs["W_o"], np.float32),
                "b_o": np.ascontiguousarray(inputs["b_o"], np.float32),
                "base_const": bconst,
            }
        )
    res = run_bass_kernel_spmd(nc, in_maps, core_ids=list(range(BS)))
    return np.stack([res.results[b]["out"] for b in range(BS)], axis=0)


# revision 41
# speedup vs baseline: 1.1630x; 1.1630x over previous
"""Trainium2 Bass kernel for Deformable-DETR multi-scale deformable attention.

8 NeuronCores, data-parallel over batch (1 batch element per core, SPMD, no
collectives).

Per core:
  Loop1 (hoisted, runs from t=0 concurrent with phase 1): offset/attention
    projections on PE, softmax + sampling locations + masked corner weights
    + unit indices on DVE/ACT for ALL query groups.
  Phase 1: V = X @ W_v + b_v on PE (fp16 in, fp32 PSUM), stored to DRAM
    "pair tables": per (head, level), unit u = e*W + x holds rows (e-1, e)
    of column x as [2, 32ch] fp16 (128 B). A bilinear 2x2x32 patch is then
    2 consecutive units -> 2 contiguous 128B descriptors per sample.
    Table writes alternate sync/scalar HWDGE queues to halve queue latency.
  Loop2 (per 128-query group): per-slot indirect DMA gathers (128
    descriptors each, the issue-rate bottleneck), fp16 broadcast-multiply +
    add-tree combine on DVE, output projection on PE.

All potentially-junk table reads (x-wrap at row ends, out-of-range
samples) are zero-masked via the per-corner validity folded into the
bilinear weights; every reachable table byte is written or zero-filled so
junk stays finite.
"""

import sys

sys.path.insert(0, "/opt/trn_rl_repo")

import dataclasses
import math

import numpy as np

import concourse.bass as bass
import concourse.mybir as mybir
import concourse.tile as tile
from concourse import bacc
from concourse.bass_utils import run_bass_kernel_spmd
from concourse.masks import make_identity

# ---------------- problem constants (hardcoded) ----------------
SPATIAL = [(100, 150), (50, 75), (25, 38), (13, 19)]
TOTAL = sum(h * w for h, w in SPATIAL)  # 19947
BS, Q, D, NH, NL, NP = 8, 900, 256, 8, 4, 4
DH = D // NH  # 32
NQG = math.ceil(Q / 128)  # 8 query groups (7x128 + 4)
MAGIC = 12582912.0  # 1.5 * 2**23: float32 round-to-nearest-int trick
LEFTOVER = False  # slot-major tail-query fast path (crashed on HW; disabled)

FP32 = mybir.dt.float32
FP16 = mybir.dt.float16
INT32 = mybir.dt.int32
ADD = mybir.AluOpType.add
SUB = mybir.AluOpType.subtract
MUL = mybir.AluOpType.mult
MAXOP = mybir.AluOpType.max
MINOP = mybir.AluOpType.min

LVL_START = []
_s = 0
for _h, _w in SPATIAL:
    LVL_START.append(_s)
    _s += _h * _w
LVL_NT = [math.ceil(h * w / 128) for h, w in SPATIAL]
LVL_T0 = [sum(LVL_NT[:i]) for i in range(NL)]
NT_TOT = sum(LVL_NT)  # 158

# Table regions in units (1 unit = 64 fp16 = [2 rows, 32 ch] of one column)
# One DRAM table per level so gathers of a level only depend on that level's
# writes (lets small levels' gathers start while level 0 is still building).
GUARD = 64
TAILG = 384  # absorbs the last head's masked x/y-slop reads
# +1 virtual tile per head: units [nt*128, (nt+1)*128) hold [v[u-W] | 0] so
# valid bottom-row (y0 = H-1) A-corners past nt*128 read real data.
REG = [(LVL_NT[_li] + 1) * 128 for _li in range(NL)]
# masked slack reads past a head's region land in the next head's written
# units (finite, zero-weighted) -- only the last head needs the tail guard.
BASE = [[GUARD + h * REG[li] for h in range(NH)] for li in range(NL)]  # [l][h]
U_L = [GUARD + NH * REG[li] + TAILG for li in range(NL)]
# xf16 staging layout: per-level, padded to 1024-row bands
LVL_NB = [math.ceil(nt * 128 / 1024) for nt in LVL_NT]
XB = [sum(LVL_NB[:i]) * 1024 for i in range(NL)]
XROWS = sum(LVL_NB) * 1024


def _np_base_const() -> np.ndarray:
    """[128, 128] f32; free position (h,l,p) -> BASE[l][h] + W_l (level-local)."""
    c = np.zeros((128,), np.float32)
    for h in range(NH):
        for li in range(NL):
            for p in range(NP):
                c[(h * NL + li) * NP + p] = BASE[li][h] + SPATIAL[li][1]
    return np.broadcast_to(c, (128, 128)).copy()


def _bc(ap, dim, count):
    """Insert a broadcast (step-0) dim at position `dim` of an AP."""
    new = list(ap.ap)
    new.insert(dim, [0, count])
    return dataclasses.replace(ap, ap=new)


def _indirect_gather_q(gp, out, in_, offset_ap, queue_num: int):
    """indirect_dma_start (gather direction only) with SWDGE queue select."""
    out_ap = gp.lower_ap_dma(out, for_indirect_dma=True)
    in_ap = gp.lower_ap_dma(in_, for_indirect_dma=True)
    assert len(in_ap) == 1 and len(out_ap) == 1
    off_ap = gp.lower_ap_dma(offset_ap)
    assert len(off_ap) == 1
    in_ap.append(off_ap[0])
    ap_shape = in_.shape
    coef = 1
    for i in range(1, len(ap_shape)):
        coef *= ap_shape[i]
    in_ap[0].dynamic_ap_info = mybir.DynamicAccessPatternInfo(
        c=0,
        actual_ap=out.ap,
        indirect_dim_max_index=ap_shape[0],
        offset_expr=[
            mybir.DynamicAccessPatternOffsetExpr(
                coef=coef,
                aff_expr=mybir.DynamicAccessPatternOffsetExprAffExpr(
                    kind="IndirectArgId", arg_id=1
                ),
            )
        ],
    )
    return gp.add_instruction(
        mybir.InstDMACopy(
            name=gp.bass.get_next_instruction_name(),
            queue=f"qPoolDynamic{queue_num or ''}",
            mode="Copy",
            ins=in_ap,
            outs=out_ap,
            oob_is_err=True,
            cce_op=mybir.AluOpType.bypass,
        )
    )


def build(reps: int = 1, ablate: set | None = None):
    ablate = ablate or set()
    nc = bacc.Bacc(
        "TRN2", target_bir_lowering=False, debug=False, num_devices=8,
    )

    t_q = nc.dram_tensor("query", [Q, D], FP32, kind="ExternalInput")
    t_rp = nc.dram_tensor("reference_points", [Q, NL * 2], FP32, kind="ExternalInput")
    t_x = nc.dram_tensor("input_flatten", [TOTAL, D], FP32, kind="ExternalInput")
    t_woff = nc.dram_tensor("W_off", [D, D], FP32, kind="ExternalInput")
    t_boff = nc.dram_tensor("b_off", [D], FP32, kind="ExternalInput")
    t_watt = nc.dram_tensor("W_attn", [D, 128], FP32, kind="ExternalInput")
    t_batt = nc.dram_tensor("b_attn", [128], FP32, kind="ExternalInput")
    t_wv = nc.dram_tensor("W_v", [D, D], FP32, kind="ExternalInput")
    t_bv = nc.dram_tensor("b_v", [D], FP32, kind="ExternalInput")
    t_wo = nc.dram_tensor("W_o", [D, D], FP32, kind="ExternalInput")
    t_bo = nc.dram_tensor("b_o", [D], FP32, kind="ExternalInput")
    t_bconst = nc.dram_tensor("base_const", [128, 128], FP32, kind="ExternalInput")
    t_out = nc.dram_tensor("out", [Q, D], FP32, kind="ExternalOutput")

    with tile.TileContext(nc) as tc:
        with (
            tc.tile_pool(name="dram", bufs=1, space="DRAM") as dpool,
            tc.tile_pool(name="const", bufs=1) as cpool,
            tc.tile_pool(name="psum", bufs=2, space="PSUM") as ppool,
            tc.tile_pool(name="psum2", bufs=1, space="PSUM") as ppool2,
        ):
            tables = []
            for _tli in range(NL):
                _tab = dpool.tile([U_L[_tli], 64], FP16, tag=f"table{_tli}")
                tables.append(_tab)

            # ---- constants / weights ----
            ident = cpool.tile([128, 128], FP32)
            make_identity(nc, ident[:])
            ident16 = cpool.tile([128, 128], FP16)
            nc.vector.tensor_copy(ident16[:], ident[:])
            # Per-level shift matrices for the table A-half (v shifted by W):
            # Sh[k, m] = [m == k + k1], Sl[k, m] = [m == k - (128 - k1)].
            shmat, slmat = {}, {}
            for _li in range(NL):
                _k1 = SPATIAL[_li][1] % 128
                _sh = cpool.tile([128, 128], FP16, tag=f"sh{_li}")
                _sl = cpool.tile([128, 128], FP16, tag=f"sl{_li}")
                nc.vector.memset(_sh[:], 0.0)
                nc.vector.memset(_sl[:], 0.0)
                nc.vector.tensor_copy(_sh[:, _k1:128], ident16[:, 0 : 128 - _k1])
                nc.vector.tensor_copy(_sl[:, 0:_k1], ident16[:, 128 - _k1 : 128])
                shmat[_li], slmat[_li] = _sh, _sl
            wv16 = cpool.tile([128, 2, D], FP16)
            wo16 = cpool.tile([128, 2, D], FP16)
            woff = cpool.tile([128, 2, D], FP32)
            watt = cpool.tile([128, 2, 128], FP32)
            for j in range(2):
                nc.gpsimd.dma_start(out=wv16[:, j, :], in_=t_wv[j * 128 : (j + 1) * 128, :])
                nc.gpsimd.dma_start(out=wo16[:, j, :], in_=t_wo[j * 128 : (j + 1) * 128, :])
                nc.sync.dma_start(out=woff[:, j, :], in_=t_woff[j * 128 : (j + 1) * 128, :])
                nc.sync.dma_start(out=watt[:, j, :], in_=t_watt[j * 128 : (j + 1) * 128, :])
            bv16 = cpool.tile([1, D], FP16)
            bo16 = cpool.tile([1, D], FP16)
            boff = cpool.tile([1, D], FP32)
            batt = cpool.tile([1, 128], FP32)
            nc.gpsimd.dma_start(out=bv16[:], in_=t_bv[None, :])
            nc.gpsimd.dma_start(out=bo16[:], in_=t_bo[None, :])
            nc.sync.dma_start(out=boff[:], in_=t_boff[None, :])
            nc.sync.dma_start(out=batt[:], in_=t_batt[None, :])
            ones32 = cpool.tile([1, 128], FP32)
            ones16 = cpool.tile([1, 128], FP16)
            nc.vector.memset(ones32[:], 1.0)
            nc.vector.memset(ones16[:], 1.0)
            bconst = cpool.tile([128, 128], FP32)
            nc.sync.dma_start(out=bconst[:], in_=t_bconst[:, :])

            # zero tile for table guard/slack fills
            zt = cpool.tile([128, 64], FP16)
            nc.vector.memset(zt[:], 0.0)

            # one-hot head map for the leftover-query combine matmul:
            # slot s (level-major: s = l*32 + h*4 + p) -> head h.
            # oneh[s, h] = sum_{l,p} ident[s, l*32 + h*4 + p], via two reduces.
            oneh = cpool.tile([128, NH], FP16) if LEFTOVER else None
            onehr = cpool.tile([128, NL * NH], FP16) if LEFTOVER else None
            if LEFTOVER:
             with nc.allow_low_precision(reason="0/1 one-hot sums are exact in fp16"):
                nc.vector.tensor_reduce(
                    onehr[:, :].rearrange("s (l h) -> s l h", l=NL),
                    ident16[:, :].rearrange("s (l h p) -> s l h p", l=NL, h=NH),
                    mybir.AxisListType.X, ADD,
                )
                nc.vector.tensor_reduce(
                    oneh[:, :],
                    onehr[:, :].rearrange("s (l h) -> s h l", l=NL),
                    mybir.AxisListType.X, ADD,
                )

            for rep in range(reps):
              if rep:
                  tc.no_sync_barrier()
              with (
                  tc.tile_pool(name=f"p2w{rep}", bufs=2) as wp2,
                  tc.tile_pool(name=f"p2s{rep}", bufs=1) as sp2,
                  tc.tile_pool(name=f"p2b{rep}", bufs=1) as bigp,
                  tc.tile_pool(name=f"p2p{rep}", bufs=3) as patpool,
                  tc.tile_pool(name=f"p2i{rep}", bufs=8) as ipool,
              ):
                # ======== Loop1: sampling indices + weights for ALL qgs =====
                # Runs concurrently with phase 1 (only needs query + weights).
                idx_t, cw_t = {}, {}
                idxT7_t, cwT7_t = {}, {}
                for qg in range(NQG):
                    q0 = qg * 128
                    nq = min(128, Q - q0)

                    qt32 = wp2.tile([128, D], FP32, tag="qt32")
                    nc.sync.dma_start(out=qt32[:nq, :], in_=t_q[q0 : q0 + nq, :])
                    qT = wp2.tile([128, 2, 128], FP32, tag="qT")
                    for j in range(2):
                        tp = ppool2.tile([128, 128], FP32, tag="tpsum")
                        nc.tensor.transpose(
                            tp[:, :nq], qt32[:nq, j * 128 : (j + 1) * 128], ident[:nq, :nq]
                        )
                        nc.scalar.copy(out=qT[:, j, :nq], in_=tp[:, :nq])

                    offp = ppool2.tile([128, D], FP32, tag="mmout2")
                    nc.tensor.matmul(offp[:nq, :], qT[:, 0, :nq], woff[:, 0, :], start=True, stop=False)
                    nc.tensor.matmul(offp[:nq, :], qT[:, 1, :nq], woff[:, 1, :], start=False, stop=False)
                    nc.tensor.matmul(offp[:nq, :], ones32[:, :nq], boff[:], start=False, stop=True)
                    off = sp2.tile([128, D], FP32, tag="off")
                    nc.scalar.copy(out=off[:nq, :], in_=offp[:nq, :])

                    attp = ppool2.tile([128, 128], FP32, tag="attp")
                    nc.tensor.matmul(attp[:nq, :], qT[:, 0, :nq], watt[:, 0, :], start=True, stop=False)
                    nc.tensor.matmul(attp[:nq, :], qT[:, 1, :nq], watt[:, 1, :], start=False, stop=False)
                    nc.tensor.matmul(attp[:nq, :], ones32[:, :nq], batt[:], start=False, stop=True)
                    att = sp2.tile([128, 128], FP32, tag="att")
                    nc.scalar.copy(out=att[:nq, :], in_=attp[:nq, :])

                    # softmax over (l,p)=16 per head
                    mx = sp2.tile([128, 1], FP32, tag="mx")
                    nc.vector.tensor_reduce(mx[:nq, :], att[:nq, :], mybir.AxisListType.X, MAXOP)
                    nmx = sp2.tile([128, 1], FP32, tag="nmx")
                    nc.vector.tensor_scalar_mul(nmx[:nq, :], mx[:nq, :], -1.0)
                    ex = sp2.tile([128, 128], FP32, tag="ex")
                    nc.scalar.activation(
                        ex[:nq, :], att[:nq, :], mybir.ActivationFunctionType.Exp,
                        bias=nmx[:nq, :], scale=1.0,
                    )
                    s16 = sp2.tile([128, 8], FP32, tag="s16")
                    nc.vector.tensor_reduce(
                        s16[:nq, :], ex[:nq, :].rearrange("q (h k) -> q h k", k=16),
                        mybir.AxisListType.X, ADD,
                    )
                    r16 = sp2.tile([128, 8], FP32, tag="r16")
                    nc.vector.reciprocal(r16[:nq, :], s16[:nq, :])
                    attn = sp2.tile([128, 128], FP32, tag="attn")
                    nc.vector.tensor_tensor(
                        attn[:nq, :].rearrange("q (h k) -> q h k", k=16),
                        ex[:nq, :].rearrange("q (h k) -> q h k", k=16),
                        _bc(r16[:nq, :], 2, 16),
                        MUL,
                    )

                    # reference points -> pixel bases per (l, p)
                    rxy = sp2.tile([128, 8], FP32, tag="rxy")
                    nc.sync.dma_start(out=rxy[:nq, :], in_=t_rp[q0 : q0 + nq, :])
                    refx = sp2.tile([128, 16], FP32, tag="refx")
                    refy = sp2.tile([128, 16], FP32, tag="refy")
                    for li, (H, W) in enumerate(SPATIAL):
                        nc.vector.tensor_scalar(
                            refx[:nq, li * 4 : li * 4 + 4],
                            rxy[:nq, 2 * li : 2 * li + 1].to_broadcast([nq, 4]),
                            float(W), -0.5, MUL, ADD,
                        )
                        nc.vector.tensor_scalar(
                            refy[:nq, li * 4 : li * 4 + 4],
                            rxy[:nq, 2 * li + 1 : 2 * li + 2].to_broadcast([nq, 4]),
                            float(H), -0.5, MUL, ADD,
                        )

                    xc = sp2.tile([128, 128], FP32, tag="xc")
                    yc = sp2.tile([128, 128], FP32, tag="yc")
                    off_v = off[:nq, :].rearrange(
                        "q (h l p two) -> q h l p two", h=NH, l=NL, p=NP
                    )
                    nc.vector.tensor_tensor(
                        xc[:nq, :].rearrange("q (h lp) -> q h lp", h=NH),
                        off_v[:, :, :, :, 0].rearrange("q h l p -> q h (l p)"),
                        _bc(refx[:nq, :], 1, NH),
                        ADD,
                    )
                    nc.vector.tensor_tensor(
                        yc[:nq, :].rearrange("q (h lp) -> q h lp", h=NH),
                        off_v[:, :, :, :, 1].rearrange("q h l p -> q h (l p)"),
                        _bc(refy[:nq, :], 1, NH),
                        ADD,
                    )

                    # floor via magic round + correction, then per-level clamp
                    def floor_clamp(src, tagp, hi_by_l):
                        f = sp2.tile([128, 128], FP32, tag="f" + tagp)
                        nc.vector.tensor_scalar_add(f[:nq, :], src[:nq, :], MAGIC)
                        nc.vector.tensor_scalar_sub(f[:nq, :], f[:nq, :], MAGIC)
                        g = sp2.tile([128, 128], FP32, tag="g" + tagp)
                        nc.vector.tensor_tensor(g[:nq, :], f[:nq, :], src[:nq, :], mybir.AluOpType.is_gt)
                        nc.vector.tensor_tensor(f[:nq, :], f[:nq, :], g[:nq, :], SUB)
                        fv = f[:nq, :].rearrange("q (h l p) -> q h l p", h=NH, l=NL)
                        for li in range(NL):
                            nc.vector.tensor_scalar(
                                fv[:, :, li, :], fv[:, :, li, :],
                                -2.0, float(hi_by_l[li]), MAXOP, MINOP,
                            )
                        return f

                    x0f = floor_clamp(xc, "x", [w for (h, w) in SPATIAL])
                    y0f = floor_clamp(yc, "y", [h for (h, w) in SPATIAL])

                    wx1 = sp2.tile([128, 128], FP32, tag="wx1")
                    wy1 = sp2.tile([128, 128], FP32, tag="wy1")
                    nc.vector.tensor_tensor(wx1[:nq, :], xc[:nq, :], x0f[:nq, :], SUB)
                    nc.vector.tensor_tensor(wy1[:nq, :], yc[:nq, :], y0f[:nq, :], SUB)

                    def corner_w(wf1, f, axis, n_by_l):
                        a0 = sp2.tile([128, 128], FP32, tag="a0" + axis)
                        a1 = sp2.tile([128, 128], FP32, tag="a1" + axis)
                        m = sp2.tile([128, 128], FP32, tag="m" + axis)
                        nc.vector.tensor_scalar(a0[:nq, :], wf1[:nq, :], -1.0, 1.0, MUL, ADD)
                        nc.vector.tensor_scalar(m[:nq, :], f[:nq, :], 0.0, None, mybir.AluOpType.is_ge)
                        nc.vector.tensor_tensor(a0[:nq, :], a0[:nq, :], m[:nq, :], MUL)
                        nc.vector.tensor_scalar(m[:nq, :], f[:nq, :], -1.0, None, mybir.AluOpType.is_ge)
                        nc.vector.tensor_tensor(a1[:nq, :], wf1[:nq, :], m[:nq, :], MUL)
                        fv = f[:nq, :].rearrange("q (h l p) -> q h l p", h=NH, l=NL)
                        mv = m[:nq, : NH * NP].rearrange("q (h p) -> q h p", h=NH)
                        for li in range(NL):
                            n = n_by_l[li]
                            for a, bound in ((a0, n - 1.0), (a1, n - 2.0)):
                                nc.vector.tensor_scalar(mv, fv[:, :, li, :], bound, None, mybir.AluOpType.is_le)
                                av = a[:nq, :].rearrange("q (h l p) -> q h l p", h=NH, l=NL)
                                nc.vector.tensor_tensor(av[:, :, li, :], av[:, :, li, :], mv, MUL)
                        return a0, a1

                    ax0, ax1 = corner_w(wx1, x0f, "x", [w for (h, w) in SPATIAL])
                    ay0, ay1 = corner_w(wy1, y0f, "y", [h for (h, w) in SPATIAL])
                    nc.vector.tensor_tensor(ay0[:nq, :], ay0[:nq, :], attn[:nq, :], MUL)
                    nc.vector.tensor_tensor(ay1[:nq, :], ay1[:nq, :], attn[:nq, :], MUL)

                    cw = ipool.tile([128, 128, 4], FP16, tag="cw")
                    for dx, ax in ((0, ax0), (1, ax1)):
                        for dy, ay in ((0, ay0), (1, ay1)):
                            nc.vector.tensor_tensor(
                                cw[:nq, :, 2 * dx + dy], ax[:nq, :], ay[:nq, :], MUL
                            )

                    # u = (y0+1)*W + x0 + BASE  (bconst = BASE + W)
                    uf = sp2.tile([128, 128], FP32, tag="uf")
                    ufv = uf[:nq, :].rearrange("q (h l p) -> q h l p", h=NH, l=NL)
                    yv = y0f[:nq, :].rearrange("q (h l p) -> q h l p", h=NH, l=NL)
                    for li in range(NL):
                        nc.vector.tensor_scalar(
                            ufv[:, :, li, :], yv[:, :, li, :],
                            float(SPATIAL[li][1]), None, MUL,
                        )
                    nc.vector.tensor_tensor(uf[:nq, :], uf[:nq, :], x0f[:nq, :], ADD)
                    nc.vector.tensor_tensor(uf[:nq, :], uf[:nq, :], bconst[:nq, :], ADD)
                    for li in range(NL):
                        nc.vector.tensor_scalar(
                            ufv[:, :, li, :], ufv[:, :, li, :],
                            0.0, float(U_L[li] - 2), MAXOP, MINOP,
                        )

                    idx = ipool.tile([128, 128], INT32, tag="idx")
                    nc.vector.tensor_copy(idx[:nq, :], uf[:nq, :])

                    idx_t[qg] = idx
                    cw_t[qg] = cw

                    if LEFTOVER and qg == NQG - 1 and nq <= 8:
                        # Leftover-query path: transpose indices and corner
                        # weights to slot-major (level-major slot order) so the
                        # tail queries gather 32 slots per instruction.
                        def tposeL(src, tag):
                            perm = sp2.tile([128, 128], FP32, tag="permT")
                            nc.vector.tensor_copy(
                                perm[:nq, :].rearrange("q (l h p) -> q l h p", l=NL, h=NH),
                                src[:nq, :].rearrange("q (h l p) -> q l h p", h=NH, l=NL),
                            )
                            tp7 = ppool2.tile([128, 128], FP32, tag="tpsum")
                            nc.tensor.transpose(tp7[:, :nq], perm[:nq, :], ident[:nq, :nq])
                            dst = sp2.tile([128, 8], FP32, tag="T" + tag)
                            nc.scalar.copy(out=dst[:, :nq], in_=tp7[:, :nq])
                            return dst

                        ufT = tposeL(uf, "uf")
                        idxT7 = ipool.tile([128, 8], INT32, tag="idxT7")
                        nc.vector.tensor_copy(idxT7[:, :nq], ufT[:, :nq])
                        a0xT = tposeL(ax0, "a0x")
                        a1xT = tposeL(ax1, "a1x")
                        a0yT = tposeL(ay0, "a0y")
                        a1yT = tposeL(ay1, "a1y")
                        cwT7 = ipool.tile([128, 8, 4], FP16, tag="cwT7")
                        for dx, axT in ((0, a0xT), (1, a1xT)):
                            for dy, ayT in ((0, a0yT), (1, a1yT)):
                                nc.vector.tensor_tensor(
                                    cwT7[:, :nq, 2 * dx + dy], axT[:, :nq], ayT[:, :nq], MUL
                                )
                        idxT7_t[0] = idxT7
                        cwT7_t[0] = cwT7

                # ---- zero-fill guards + unwritten slack (sync/scalar split) --
                _zq = [0]

                def zfill(tab, u0, n):
                    while n > 0:
                        k = min(n, 128)
                        eng = nc.sync if (_zq[0] & 1) == 0 else nc.scalar
                        _zq[0] += 1
                        eng.dma_start(out=tab[u0 : u0 + k, :], in_=zt[:k, :])
                        u0 += k
                        n -= k

                if rep == 0:
                    for li in (3, 2, 1, 0):
                        # head guard + tail guard; every unit in [0, nt*128)
                        # of every head is fully written by the merged A|B
                        # table writes (A-half zeros where pos-W < 0).
                        zfill(tables[li], 0, GUARD)
                        zfill(tables[li], U_L[li] - TAILG, TAILG)

                # ============ Phase 1: value projection -> pair tables ======
                CHUNK_BANDS = 3  # 24(+1) tiles per vcat chunk
                with (
                    tc.tile_pool(name=f"vsb{rep}", bufs=2) as vpool,
                    tc.tile_pool(name=f"p1w{rep}", bufs=3) as wp1,
                    tc.tile_pool(name=f"p1d{rep}", bufs=1, space="DRAM") as dp1,
                ):
                    if "phase1" not in ablate:
                        # per-level fp16 copy of X in DRAM (cast during DMA),
                        # processed smallest level first so its tables finish
                        # early and loop2 gathers can start.
                        xf16 = dp1.tile([XROWS, D], FP16)
                        zrow = wp1.tile([128, D], FP16, tag="zrow")
                        nc.vector.memset(zrow[:, :], 0.0)
                        for li in (3, 2, 1, 0):
                            H, W = SPATIAL[li]
                            npos = H * W
                            p0 = XB[li]
                            k1 = W % 128  # partition shift of the A-half
                            tb = W // 128  # whole-tile shift of the A-half
                            nc.gpsimd.dma_start(
                                out=xf16[p0 : p0 + npos, :],
                                in_=t_x[LVL_START[li] : LVL_START[li] + npos, :],
                            )
                            r = p0 + npos
                            while r < p0 + LVL_NB[li] * 1024:
                                k = min(128, p0 + LVL_NB[li] * 1024 - r)
                                nc.scalar.dma_start(out=xf16[r : r + k, :], in_=zrow[:k, :])
                                r += k
                            prev_vcat, prev_tc0 = None, -1
                            for c0 in range(0, LVL_NB[li], CHUNK_BANDS):
                                cbands = min(CHUNK_BANDS, LVL_NB[li] - c0)
                                tc0 = c0 * 8  # first tile (level-local)
                                # +1: the virtual bottom tile (A-only) rides in
                                # the last chunk of the level.
                                is_last = c0 + cbands >= LVL_NB[li]
                                cap = cbands * 8 + (1 if is_last else 0)
                                ntc = min(cap, LVL_NT[li] + 1 - tc0)
                                assert ntc <= CHUNK_BANDS * 8 + 1
                                vcat = vpool.tile(
                                    [128, CHUNK_BANDS * 8 + 1, NH, 2 * DH], FP16, tag="vcat"
                                )

                                def bsrc(tt, lo, hi):
                                    # B-half (v) of level-local tile tt, partitions lo:hi
                                    if tt >= tc0:
                                        return vcat[lo:hi, tt - tc0, :, DH : 2 * DH]
                                    return prev_vcat[lo:hi, tt - prev_tc0, :, DH : 2 * DH]

                                for bloc in range(cbands):
                                    band = c0 + bloc
                                    xTb = wp1.tile([128, 2, 1024], FP16, tag="xTb")
                                    for j in range(2):
                                        nc.sync.dma_start_transpose(
                                            out=xTb[:, j, :],
                                            in_=xf16[p0 + band * 1024 : p0 + (band + 1) * 1024, j * 128 : (j + 1) * 128],
                                        )
                                    for tloc in range(8):
                                        t = band * 8 + tloc
                                        if t >= LVL_NT[li]:
                                            break
                                        vp = ppool.tile([128, D], FP32, tag="mmout")
                                        nc.tensor.matmul(vp[:], xTb[:, 0, tloc * 128 : (tloc + 1) * 128], wv16[:, 0, :], start=True, stop=False)
                                        nc.tensor.matmul(vp[:], xTb[:, 1, tloc * 128 : (tloc + 1) * 128], wv16[:, 1, :], start=False, stop=False)
                                        nc.tensor.matmul(vp[:], ones16[:, :], bv16[:], start=False, stop=True)
                                        nc.scalar.copy(
                                            out=vcat[:, t - tc0, :, DH : 2 * DH],
                                            in_=vp[:].rearrange("p (h c) -> p h c", h=NH),
                                        )
                                        # A-half: v shifted back W positions
                                        # (unit u holds [v[u-W] | v[u]]).
                                        pa = ppool.tile([128, D], FP32, tag="mmoutA")
                                        if t - tb >= 0:
                                            more = t - tb - 1 >= 0
                                            nc.tensor.matmul(
                                                pa[:, :], shmat[li][:, :],
                                                bsrc(t - tb, 0, 128),
                                                start=True, stop=not more,
                                            )
                                            if more:
                                                nc.tensor.matmul(
                                                    pa[:, :], slmat[li][:, :],
                                                    bsrc(t - tb - 1, 0, 128),
                                                    start=False, stop=True,
                                                )
                                        else:
                                            nc.vector.memset(pa[:, :], 0.0)
                                        nc.scalar.copy(
                                            out=vcat[:, t - tc0, :, 0:DH],
                                            in_=pa[:].rearrange("p (h c) -> p h c", h=NH),
                                        )
                                t_v = LVL_NT[li]  # virtual bottom tile
                                if tc0 <= t_v < tc0 + cap:
                                    nc.vector.memset(
                                        vcat[:, t_v - tc0, :, DH : 2 * DH], 0.0
                                    )
                                    pa = ppool.tile([128, D], FP32, tag="mmoutA")
                                    nc.tensor.matmul(
                                        pa[:, :], shmat[li][:, :],
                                        bsrc(t_v - tb, 0, 128),
                                        start=True, stop=False,
                                    )
                                    nc.tensor.matmul(
                                        pa[:, :], slmat[li][:, :],
                                        bsrc(t_v - tb - 1, 0, 128),
                                        start=False, stop=True,
                                    )
                                    nc.scalar.copy(
                                        out=vcat[:, t_v - tc0, :, 0:DH],
                                        in_=pa[:].rearrange("p (h c) -> p h c", h=NH),
                                    )
                                u0 = tc0 * 128
                                for h in range(NH):
                                    b = BASE[li][h]
                                    src = vcat[:, 0:ntc, h, :]
                                    dst = tables[li][b + u0 : b + u0 + ntc * 128, :].rearrange(
                                        "(t p) c -> p t c", p=128
                                    )
                                    eng = nc.sync if (h & 1) == 0 else nc.scalar
                                    eng.dma_start(out=dst, in_=src)
                                prev_vcat, prev_tc0 = vcat, tc0

                # ============ Loop2: gather + combine + output proj =========
                # Level-0 gathers + combine for qg k are emitted after qg
                # k+1's small-level gathers, so the (late-finishing) level-0
                # table writes get an extra ~130us of gather work as cover.
                patches_t = {}

                def emit_smalls(qg):
                    q0 = qg * 128
                    nq = min(128, Q - q0)
                    idx = idx_t[qg]
                    # ---- gather: one 256B patch per partition per instr ----
                    # (HW indirect DMA semantics: one index per partition,
                    #  contiguous out-row-sized read.)
                    patches = patpool.tile([128, 16384], FP16, tag="patches")
                    patches_t[qg] = patches
                    pview = patches[:nq, :].rearrange("q (s e) -> q s e", e=128)
                    if "gather" in ablate:
                        nc.vector.memset(patches[:, :], 0.0)
                        return
                    for li in (3, 2, 1):
                        for h in range(NH):
                            for p in range(NP):
                                s = (h * NL + li) * NP + p
                                _indirect_gather_q(
                                    nc.gpsimd,
                                    pview[:, s, :],
                                    tables[li][:, :],
                                    idx[:nq, s : s + 1],
                                    queue_num=0,
                                )

                def emit_l0_combine(qg):
                    q0 = qg * 128
                    nq = min(128, Q - q0)
                    idx = idx_t[qg]
                    cw = cw_t[qg]
                    patches = patches_t.pop(qg)
                    pview = patches[:nq, :].rearrange("q (s e) -> q s e", e=128)
                    if "gather" not in ablate:
                        li = 0
                        for h in range(NH):
                            for p in range(NP):
                                s = (h * NL + li) * NP + p
                                _indirect_gather_q(
                                    nc.gpsimd,
                                    pview[:, s, :],
                                    tables[li][:, :],
                                    idx[:nq, s : s + 1],
                                    queue_num=0,
                                )

                    # ---- combine ----
                    if "combine" in ablate:
                        og = wp2.tile([128, D], FP16, tag="og")
                        nc.vector.memset(og[:, :], 0.0)
                    if "combine" not in ablate:
                        # weight multiply in place (out == in1, elementwise)
                        nc.vector.tensor_tensor(
                            patches[:nq, :].rearrange("q (s c ch) -> q s c ch", s=128, c=4),
                            patches[:nq, :].rearrange("q (s c ch) -> q s c ch", s=128, c=4),
                            _bc(cw[:nq, :, :], 3, DH),
                            MUL,
                        )
                        # add-tree fully in place inside patches (out == in1
                        # elementwise), so the tile frees 31KB of tree scratch
                        # and patches gets a 3-deep ring.
                        pmv = patches[:nq, :].rearrange("q (s d) -> q s d", d=128)
                        nc.vector.tensor_tensor(pmv[:, :, 0:64], pmv[:, :, 0:64], pmv[:, :, 64:128], ADD)
                        nc.vector.tensor_tensor(pmv[:, :, 0:32], pmv[:, :, 0:32], pmv[:, :, 32:64], ADD)
                        sv = patches[:nq, :].rearrange("q (h k d) -> q h k d", h=NH, k=16)
                        nc.vector.tensor_tensor(sv[:, :, 0:8, 0:32], sv[:, :, 0:8, 0:32], sv[:, :, 8:16, 0:32], ADD)
                        nc.vector.tensor_tensor(sv[:, :, 0:4, 0:32], sv[:, :, 0:4, 0:32], sv[:, :, 4:8, 0:32], ADD)
                        nc.vector.tensor_tensor(sv[:, :, 0:2, 0:32], sv[:, :, 0:2, 0:32], sv[:, :, 2:4, 0:32], ADD)
                        og = wp2.tile([128, D], FP16, tag="og")
                        if nq < 128:
                            nc.vector.memset(og[:, :], 0.0)
                        nc.vector.tensor_tensor(
                            og[:nq, :].rearrange("q (h ch) -> q h ch", h=NH),
                            sv[:, :, 0, 0:32], sv[:, :, 1, 0:32], ADD,
                        )

                    # ---- output projection ----
                    ogT = wp2.tile([128, 2, 128], FP16, tag="ogT")
                    for j in range(2):
                        nc.scalar.dma_start_transpose(
                            out=ogT[:, j, :], in_=og[:, j * 128 : (j + 1) * 128]
                        )
                    outp = ppool2.tile([128, D], FP32, tag="mmout2")
                    nc.tensor.matmul(outp[:nq, :], ogT[:, 0, :nq], wo16[:, 0, :], start=True, stop=False)
                    nc.tensor.matmul(outp[:nq, :], ogT[:, 1, :nq], wo16[:, 1, :], start=False, stop=False)
                    nc.tensor.matmul(outp[:nq, :], ones16[:, :nq], bo16[:], start=False, stop=True)
                    ofin = wp2.tile([128, D], FP32, tag="ofin")
                    nc.scalar.copy(out=ofin[:nq, :], in_=outp[:nq, :])
                    nc.scalar.dma_start(out=t_out[q0 : q0 + nq, :], in_=ofin[:nq, :])

                def emit_leftover(qg):
                    # Tail queries (nq <= 8): slot-major gathers -- one
                    # instruction per (query, level) with slots on partitions,
                    # combine via one-hot head matmul + c-reduce.
                    q0 = qg * 128
                    nq = min(128, Q - q0)
                    idxT7 = idxT7_t[0]
                    cwT7 = cwT7_t[0]
                    p7 = bigp.tile([128, 8, 128], FP16, tag="p7")
                    if "gather" not in ablate:
                        for q in range(nq):
                            for li in (3, 2, 1, 0):
                                _indirect_gather_q(
                                    nc.gpsimd,
                                    p7[li * 32 : (li + 1) * 32, q, :],
                                    tables[li][:, :],
                                    idxT7[li * 32 : (li + 1) * 32, q : q + 1],
                                    queue_num=0,
                                )
                    else:
                        nc.vector.memset(p7[:, :, :], 0.0)
                    # weight multiply in place: [s, q, c, ch] x cwT[s, q, c]
                    nc.vector.tensor_tensor(
                        p7[:, 0:nq, :].rearrange("s q (c ch) -> s q c ch", c=4),
                        p7[:, 0:nq, :].rearrange("s q (c ch) -> s q c ch", c=4),
                        _bc(cwT7[:, 0:nq, :], 3, DH),
                        MUL,
                    )
                    # sum over slots per head: psum[h, (q c ch)]
                    ps7 = ppool2.tile([128, 512], FP32, tag="ps7")
                    nc.tensor.matmul(
                        ps7[:NH, : nq * 128],
                        oneh[:, :],
                        p7[:, 0:nq, :].rearrange("s q e -> s (q e)"),
                        start=True, stop=True,
                    )
                    # reduce the 4 corners: stage[h, q, ch]
                    st7 = sp2.tile([128, 8 * DH], FP16, tag="st7")
                    with nc.allow_low_precision(
                        reason="4-corner fp16 sum matches the main path's fp16 add-tree"
                    ):
                        nc.vector.tensor_reduce(
                            st7[:NH, : nq * DH].rearrange("h (q ch) -> h q ch", ch=DH),
                            ps7[:NH, : nq * 128].rearrange(
                                "h (q c ch) -> h q ch c", c=4, ch=DH
                            ),
                            mybir.AxisListType.X, ADD,
                        )
                    og = wp2.tile([128, D], FP16, tag="og")
                    nc.vector.memset(og[:, :], 0.0)
                    for q in range(nq):
                        # partition-fold: [8 heads, 32ch] -> one 256-wide row
                        nc.sync.dma_start(
                            out=og[q : q + 1, :].rearrange("o (h ch) -> o h ch", h=NH),
                            in_=st7[:NH, q * DH : (q + 1) * DH],
                        )
                    # ---- output projection (same as main path) ----
                    ogT = wp2.tile([128, 2, 128], FP16, tag="ogT")
                    for j in range(2):
                        nc.scalar.dma_start_transpose(
                            out=ogT[:, j, :], in_=og[:, j * 128 : (j + 1) * 128]
                        )
                    outp = ppool2.tile([128, D], FP32, tag="mmout2")
                    nc.tensor.matmul(outp[:nq, :], ogT[:, 0, :nq], wo16[:, 0, :], start=True, stop=False)
                    nc.tensor.matmul(outp[:nq, :], ogT[:, 1, :nq], wo16[:, 1, :], start=False, stop=False)
                    nc.tensor.matmul(outp[:nq, :], ones16[:, :nq], bo16[:], start=False, stop=True)
                    ofin = wp2.tile([128, D], FP32, tag="ofin")
                    nc.scalar.copy(out=ofin[:nq, :], in_=outp[:nq, :])
                    nc.scalar.dma_start(out=t_out[q0 : q0 + nq, :], in_=ofin[:nq, :])

                pend = []
                nmain = NQG - 1 if LEFTOVER else NQG
                for qg in range(nmain):
                    emit_smalls(qg)
                    pend.append(qg)
                    if len(pend) > 1:
                        emit_l0_combine(pend.pop(0))
                while pend:
                    emit_l0_combine(pend.pop(0))
                if LEFTOVER:
                    emit_leftover(NQG - 1)

    nc.compile()
    return nc


_NC_CACHE = None


def kernel(**inputs) -> np.ndarray:
    global _NC_CACHE
    if _NC_CACHE is None:
        _NC_CACHE = build()
    nc = _NC_CACHE
    bconst = _np_base_const()
    in_maps = []
    for b in range(BS):
        in_maps.append(
            {
                "query": np.ascontiguousarray(inputs["query"][b], np.float32),
                "reference_points": np.ascontiguousarray(
                    inputs["reference_points"][b], np.float32
                ).reshape(Q, NL * 2),
                "input_flatten": np.ascontiguousarray(inputs["input_flatten"][b], np.float32),
                "W_off": np.ascontiguousarray(inputs["W_off"], np.float32),
                "b_off": np.ascontiguousarray(inputs["b_off"], np.float32),
                "W_attn": np.ascontiguousarray(inputs["W_attn"], np.float32),
                "b_attn": np.ascontiguousarray(inputs["b_attn"], np.float32),
                "W_v": np.ascontiguousarray(inputs["W_v"], np.float32),
                "b_v": np.ascontiguousarray(inputs["b_v"], np.float32),
                "W_o": np.ascontiguousarray(inputs["W_o"], np.float32),
                "b_o": np.ascontiguousarray(inputs["b_o"], np.float32),
                "base_const": bconst,
            }
        )
    res = run_bass_kernel_spmd(nc, in_maps, core_ids=list(range(BS)))
    return np.stack([res.results[b]["out"] for b in range(BS)], axis=0)
